# revision 26
# baseline (speedup 1.0000x reference)
# MoE layer (top-2 routing, degenerate capacity C=2) on 8 TRN2 NeuronCores.
#
# Math (the reference collapses the capacity axis: both slots carry identical
# values, so the combine contributes a factor 2):
#   scores = softmax(x @ Wg + bg)                      [G,S,E]
#   top-2 per token -> dm (0/1 mask), cw = 2 * softmax(top2 probs) scattered
#   D[e,g,:]  = sum_s dm[g,s,e] * x[g,s,:]             (dispatch, per group)
#   h[e,g,:]  = silu(D[e,g,:] @ wi[e].T)
#   eo[e,g,:] = h[e,g,:] @ wo[e].T
#   out[g,s,:] = 2 * sum_e cw[g,s,e] * eo[e,g,:]
#
# Sharding: core c owns group g=c for gating/dispatch/combine and expert e=c
# for the FFN; two small AllToAlls (E==G==n_cores==8) move the [8,M]-sized
# dispatched rows / expert outputs between the two phases.
#
# Perf structure (cost-model driven):
#  - Gating runs in exact fp32 (PE transposes of x + fp32 score matmuls);
#    top-2 selection is bit-compatible with the fp32 reference path.
#  - FFN weights are stored in HBM as float8_e3m4 scaled by 64/128 (halves
#    the dominant DMA traffic); they are upcast fp8->bf16 on-chip by the
#    DVE/Act/Pool engines (cheap there), and all matmuls run in bf16.
#    The scales fold into the silu (scale=1/64) and the combine weights
#    (cw *= 2/8192), costing zero extra instructions.
#  - x is loaded once (fp32); the bf16 copy for the dispatch matmul is cast
#    on-chip instead of a second HBM read.
#  - Weight DMAs are 8 (wi) + 4 (wo) giant stripe transfers with contiguous
#    per-partition runs; interleaved with x tiles on the sync queue so the
#    DMA engines never idle.

import os
from contextlib import ExitStack

import numpy as np
import ml_dtypes

import concourse.bass as bass
from concourse import bacc
import concourse.mybir as mybir
import concourse.tile as tile
from concourse.bass import ts
from concourse.masks import make_identity

F32 = mybir.dt.float32
BF16 = mybir.dt.bfloat16
F8E3 = mybir.dt.float8e3
AF = mybir.ActivationFunctionType
ALU = mybir.AluOpType
AX = mybir.AxisListType

P = 128

G_FULL, S_FULL, M_FULL, H_FULL, E_FULL = 8, 2048, 2048, 8192, 8
N_CORES = 8

SCALE_WI = 64.0       # host multiplies wi by this before fp8 quantization
SCALE_WO = 128.0      # host multiplies wo by this before fp8 quantization
CW_SCALE = 2.0 / (SCALE_WI * SCALE_WO)   # capacity factor 2 / both scales

NSUP, HSUP = 16, 512  # wi stripes along H (one 512-wide h-chunk per stripe)
MC, MCH = 4, 512      # FFN2 / combine / output m-chunks

LAST_RESULT = None  # BassKernelResults of the most recent device run (for test.py)


def build_bass(S=S_FULL, M=M_FULL, H=H_FULL, E=E_FULL, n_cores=N_CORES):
    assert E == n_cores
    SB, MO, HB = S // P, M // P, H // P
    HC = HSUP // 512

    nc = bacc.Bacc(num_devices=n_cores)
    rg = [list(range(n_cores))]

    xg = nc.declare_dram_parameter("xg", [S, M], F32, False)
    wg = nc.declare_dram_parameter("wg", [P, MO, E], F32, False)
    bgp = nc.declare_dram_parameter("bgp", [1, E], F32, False)
    wi8 = nc.declare_dram_parameter("wi8", [NSUP, P, MO, HSUP], F8E3, False)
    wo8 = nc.declare_dram_parameter("wo8", [MC, 4, P, HB // 4, MCH], F8E3, False)
    out = nc.declare_dram_parameter("out", [S, M], F32, True)

    with tile.TileContext(nc) as tc, ExitStack() as stack:
        # ---------- persistent pools ----------
        const_pool = stack.enter_context(tc.tile_pool(name="const", bufs=1))
        ident_f = const_pool.tile([P, P], F32, name="ident_f")
        make_identity(nc, ident_f)
        ident_b = const_pool.tile([P, P], BF16, name="ident_b")
        nc.vector.tensor_copy(ident_b[:], ident_f[:])
        ones1 = const_pool.tile([1, P], F32, name="ones1")
        nc.vector.memset(ones1[:], 1.0)
        wg_sb = const_pool.tile([P, MO, E], F32, name="wg_sb")
        nc.scalar.dma_start(wg_sb[:], wg[:])
        bg_sb = const_pool.tile([1, E], F32, name="bg_sb")
        nc.scalar.dma_start(bg_sb[:], bgp[:])

        keep_pool = stack.enter_context(tc.tile_pool(name="keep", bufs=1))
        cwT_sb = keep_pool.tile([E, SB, P], BF16, name="cwT_sb")
        dt_sb = keep_pool.tile([P, MO, E], BF16, name="dt_sb")
        ht_sb = keep_pool.tile([P, HB, E], BF16, name="ht_sb")

        # weight streaming pools (created early; DMAs emitted at chosen points)
        wo8_pool = stack.enter_context(tc.tile_pool(name="wo8", bufs=12))
        wi8_ctx = tc.tile_pool(name="wi8", bufs=2)
        wi8_pool = wi8_ctx.__enter__()

        dram = stack.enter_context(tc.tile_pool(name="dram", bufs=1, space="DRAM"))
        d_in = [dram.tile([E, M], BF16, name=f"d_in{i}") for i in range(2)]
        d_out = [dram.tile([E, M], BF16, name=f"d_out{i}") for i in range(2)]
        eo_in = [dram.tile([E, MCH], BF16, name=f"eo_in{i}") for i in range(MC)]
        eo_out = [dram.tile([E, MCH], BF16, name=f"eo_out{i}") for i in range(MC)]

        wi8_t = {}

        def fetch_wi(hs):
            t = wi8_pool.tile([P, MO, HSUP], F8E3, tag="wi8", name=f"wi8_{hs}")
            nc.sync.dma_start(t[:], wi8[hs, :, :, :])
            wi8_t[hs] = t

        wo8_t = {}

        def fetch_wo(mc, q):
            t = wo8_pool.tile([P, HB // 4, MCH], F8E3, tag="wo8", name=f"wo8_{mc}_{q}")
            nc.sync.dma_start(t[:], wo8[mc, q, :, :, :])
            wo8_t[(mc, q)] = t

        # ---------- phase A: gating + dispatch (group-parallel) ----------
        with (
            tc.tile_pool(name="xa", bufs=3) as xa,
            tc.tile_pool(name="xt", bufs=2) as xt,
            tc.tile_pool(name="sp", bufs=2) as sp,
            tc.tile_pool(name="psA", bufs=2, space="PSUM") as psA,
            tc.tile_pool(name="psC", bufs=2, space="PSUM") as psC,
            tc.tile_pool(name="psD", bufs=1, space="PSUM") as psD,
        ):
            d_ps = psD.tile([E, M], F32, name="d_ps")
            # warmups: absorb const-pool ticks into PE's vector clock so the
            # hot-loop matmuls carry few semaphore waits each.
            ptd = psA.tile([P, 4 * P], F32, tag="pt", bufs=2, name="ptd")
            nc.tensor.transpose(ptd[:, 0:P], ident_f[:], ident_f[:])
            dmy0 = psA.tile([E, E], F32, tag="score", bufs=1, name="dmy0")
            nc.tensor.matmul(dmy0[:], lhsT=wg_sb[:, 0, :], rhs=wg_sb[:, 0, :], start=True, stop=True)
            dmyc = psC.tile([P, P], BF16, tag="pc", bufs=1, name="dmyc")
            nc.tensor.transpose(dmyc[:], ident_b[:], ident_b[:])

            for sb in range(SB):
                x_t = xa.tile([P, M], F32, tag="x", name=f"x{sb}")
                nc.sync.dma_start(x_t[:], xg[ts(sb, P), :])
                xT_t = xt.tile([P, M], F32, tag="xT", name=f"xT{sb}")
                for q in range(4):
                    pt = psA.tile([P, 4 * P], F32, tag="pt", bufs=2, name=f"pt{sb}_{q}")
                    for j in range(4):
                        mo = q * 4 + j
                        nc.tensor.transpose(pt[:, ts(j, P)], x_t[:, ts(mo, P)], ident_f[:])
                    if q % 2 == 0:
                        nc.vector.tensor_copy(xT_t[:, ts(q, 4 * P)], pt[:])
                    else:
                        nc.scalar.copy(xT_t[:, ts(q, 4 * P)], pt[:])
                # bf16 x for the dispatch matmul (cast on-chip; saves an HBM read)
                x_b = xa.tile([P, M], BF16, tag="xb", name=f"xb{sb}")
                if sb % 2 == 0:
                    nc.vector.tensor_copy(x_b[:], x_t[:])
                else:
                    nc.gpsimd.tensor_copy(x_b[:], x_t[:])

                score_ps = psA.tile([P, E], F32, tag="score", bufs=1, name=f"score{sb}")
                for mo in range(MO):
                    nc.tensor.matmul(
                        score_ps[:], lhsT=xT_t[:, ts(mo, P)], rhs=wg_sb[:, mo, :],
                        start=(mo == 0), stop=False,
                    )
                nc.tensor.matmul(
                    score_ps[:], lhsT=ones1[:], rhs=bg_sb[:], start=False, stop=True,
                )

                # top-2 gating in [tokens, E] layout (fp32 throughout)
                rm = sp.tile([P, 1], F32, tag="rm", name=f"rm{sb}")
                nc.vector.tensor_reduce(rm[:], score_ps[:], axis=AX.X, op=ALU.max, negate=True)
                probs = sp.tile([P, E], F32, tag="probs", name=f"probs{sb}")
                sume = sp.tile([P, 1], F32, tag="sume", name=f"sume{sb}")
                nc.scalar.activation(probs[:], score_ps[:], AF.Exp, bias=rm[:], accum_out=sume[:])
                rcp = sp.tile([P, 1], F32, tag="rcp", name=f"rcp{sb}")
                nc.vector.reciprocal(rcp[:], sume[:])
                pn = sp.tile([P, E], F32, tag="pn", name=f"pn{sb}")
                nc.vector.tensor_scalar_mul(pn[:], probs[:], rcp[:])
                p1 = sp.tile([P, 1], F32, tag="p1", name=f"p1{sb}")
                nc.vector.tensor_reduce(p1[:], pn[:], axis=AX.X, op=ALU.max)
                oh1 = sp.tile([P, E], F32, tag="oh1", name=f"oh1{sb}")
                nc.vector.tensor_scalar(oh1[:], pn[:], p1[:], None, op0=ALU.is_equal)
                pm = sp.tile([P, E], F32, tag="pm", name=f"pm{sb}")
                nc.gpsimd.tensor_tensor(pm[:], pn[:], oh1[:], ALU.subtract)
                p2 = sp.tile([P, 1], F32, tag="p2", name=f"p2{sb}")
                nc.vector.tensor_reduce(p2[:], pm[:], axis=AX.X, op=ALU.max)
                oh2 = sp.tile([P, E], F32, tag="oh2", name=f"oh2{sb}")
                nc.vector.tensor_scalar(oh2[:], pm[:], p2[:], None, op0=ALU.is_equal)
                e1 = sp.tile([P, 1], F32, tag="e1", name=f"e1{sb}")
                nc.scalar.activation(e1[:], p1[:], AF.Exp)
                e2 = sp.tile([P, 1], F32, tag="e2", name=f"e2{sb}")
                nc.scalar.activation(e2[:], p2[:], AF.Exp)
                s12 = sp.tile([P, 1], F32, tag="s12", name=f"s12{sb}")
                nc.vector.tensor_tensor(s12[:], e1[:], e2[:], ALU.add)
                r12 = sp.tile([P, 1], F32, tag="r12", name=f"r12{sb}")
                nc.vector.reciprocal(r12[:], s12[:])
                # CW_SCALE folds the capacity factor 2 and both weight scales
                w1 = sp.tile([P, 1], F32, tag="w1", name=f"w1{sb}")
                nc.vector.tensor_scalar(w1[:], e1[:], r12[:], CW_SCALE, op0=ALU.mult, op1=ALU.mult)
                w2 = sp.tile([P, 1], F32, tag="w2", name=f"w2{sb}")
                nc.vector.tensor_scalar(w2[:], e2[:], r12[:], CW_SCALE, op0=ALU.mult, op1=ALU.mult)
                cw_t = sp.tile([P, E], F32, tag="cw", name=f"cw{sb}")
                nc.vector.tensor_scalar_mul(cw_t[:], oh1[:], w1[:])
                t2 = sp.tile([P, E], F32, tag="t2", name=f"t2{sb}")
                nc.vector.tensor_scalar_mul(t2[:], oh2[:], w2[:])
                nc.vector.tensor_tensor(cw_t[:], cw_t[:], t2[:], ALU.add)
                # dispatch mask in bf16 (values 0/1 exact)
                dm_b = sp.tile([P, E], BF16, tag="dmb", name=f"dmb{sb}")
                nc.gpsimd.tensor_tensor(dm_b[:], oh1[:], oh2[:], ALU.add)

                # cw^T (bf16) into [E, S] layout for the combine matmul
                cw_b = sp.tile([P, E], BF16, tag="cwb", name=f"cwb{sb}")
                nc.vector.tensor_copy(cw_b[:], cw_t[:])
                pc = psC.tile([P, P], BF16, tag="pc", bufs=1, name=f"pc{sb}")
                nc.tensor.transpose(pc[:E, :], cw_b[:], ident_b[:])
                nc.vector.tensor_copy(cwT_sb[:, sb, :], pc[:E, :])

                # dispatch: D[e,m] += dm[s,e]^T @ x[s,m]  (bf16, accumulated over sb)
                for c in range(4):
                    nc.tensor.matmul(
                        d_ps[:, ts(c, MCH)],
                        lhsT=dm_b[:],
                        rhs=x_b[:, ts(c, MCH)],
                        start=(sb == 0), stop=(sb == SB - 1),
                    )

            # queue weight stripes behind the x tiles; mc0's wo quarters fill
            # the DMA gap while later wi stripes wait for their pool slots
            # (freed only as FFN1 casts consume them).
            for hs in range(4):
                fetch_wi(hs)
            for q in range(4):
                fetch_wo(0, q)

            # dispatch AllToAll: row e -> core e; receive [G, M] for my expert
            d_sb = sp.tile([E, M], BF16, tag="dsb", bufs=1, name="d_sb")
            nc.vector.tensor_copy(d_sb[:, 0:M // 2], d_ps[:, 0:M // 2])
            nc.scalar.copy(d_sb[:, M // 2:M], d_ps[:, M // 2:M])
            nc.gpsimd.dma_start(d_in[0][:], d_sb[:])
            nc.gpsimd.collective_compute(
                "AllToAll", ALU.bypass, replica_groups=rg,
                ins=[d_in[0].opt()], outs=[d_out[0].opt()],
            )
            for hs in range(4, NSUP):
                fetch_wi(hs)
            de_b = sp.tile([E, M], BF16, tag="deb", bufs=1, name="de_b")
            nc.scalar.dma_start(de_b[:], d_out[0][:])
            for mo in range(MO):
                pd = psC.tile([P, E], BF16, tag="pc", bufs=1, name=f"pd{mo}")
                nc.tensor.transpose(pd[:], de_b[:, ts(mo, P)], ident_b[:E, :E])
                if mo % 2 == 0:
                    nc.vector.tensor_copy(dt_sb[:, mo, :], pd[:])
                else:
                    nc.scalar.copy(dt_sb[:, mo, :], pd[:])

        for mc in range(MC):
            for q in range(4):
                if (mc, q) not in wo8_t:
                    fetch_wo(mc, q)

        # ---------- phase B1: expert FFN1 + silu (expert-parallel) ----------
        cast_rr = 0

        def cast(dst, src):
            nonlocal cast_rr
            eng = (nc.vector.tensor_copy, nc.scalar.copy, nc.gpsimd.tensor_copy)[cast_rr % 3]
            cast_rr += 1
            eng(dst, src)

        with (
            tc.tile_pool(name="wib", bufs=4) as wib_pool,
            tc.tile_pool(name="sp2", bufs=2) as sp2,
            tc.tile_pool(name="psB", bufs=3, space="PSUM") as psB,
            tc.tile_pool(name="psH", bufs=2, space="PSUM") as psH,
        ):
            dmy1 = psB.tile([E, E], F32, tag="psh", name="dmy1")
            nc.tensor.matmul(dmy1[:], lhsT=dt_sb[:, MO - 1, :], rhs=dt_sb[:, MO - 1, :], start=True, stop=True)
            FFN1_ENG = [0, 1, 2, 0, 2, 1, 0, 2, 0, 1, 2, 0, 1, 2, 0, 0]
            for hs in range(NSUP):
                wib_t = wib_pool.tile([P, MO, HSUP], BF16, tag="wib", name=f"wib{hs}")
                for mo in range(MO):
                    eng = (nc.vector.tensor_copy, nc.scalar.copy, nc.gpsimd.tensor_copy)[FFN1_ENG[mo]]
                    eng(wib_t[:, mo, :], wi8_t[hs][:, mo, :])
                hc = hs
                ps_h = psB.tile([E, MCH], F32, tag="psh", name=f"psh{hc}")
                for mo in range(MO):
                    nc.tensor.matmul(
                        ps_h[:], lhsT=dt_sb[:, mo, :],
                        rhs=wib_t[:, mo, :],
                        start=(mo == 0), stop=(mo == MO - 1),
                    )
                hf = sp2.tile([E, MCH], F32, tag="hf", name=f"hf{hc}")
                nc.vector.tensor_copy(hf[:], ps_h[:])
                sg = sp2.tile([E, MCH], F32, tag="sg", name=f"sg{hc}")
                nc.scalar.activation(sg[:], hf[:], AF.Sigmoid, scale=1.0 / SCALE_WI)
                h_sb = sp2.tile([E, MCH], BF16, tag="hsb", name=f"h{hc}")
                nc.gpsimd.tensor_tensor(h_sb[:], hf[:], sg[:], ALU.mult)
                pht = psH.tile([P, 4, E], BF16, tag="pht", name=f"pht{hc}")
                for j in range(4):
                    nc.tensor.transpose(pht[:, j, :], h_sb[:, ts(j, P)], ident_b[:E, :E])
                if hc % 2 == 0:
                    nc.vector.tensor_copy(ht_sb[:, 4 * hc:4 * (hc + 1), :], pht[:])
                else:
                    nc.scalar.copy(ht_sb[:, 4 * hc:4 * (hc + 1), :], pht[:])

        wi8_ctx.__exit__(None, None, None)

        # ---------- phase B2: FFN2 + AllToAll + combine, pipelined per m-chunk ----------
        with (
            tc.tile_pool(name="wob", bufs=20) as wob_pool,
            tc.tile_pool(name="sp3", bufs=3) as sp3,
            tc.tile_pool(name="outp", bufs=4) as outp,
            tc.tile_pool(name="psE", bufs=2, space="PSUM") as psE,
            tc.tile_pool(name="psO", bufs=3, space="PSUM") as psO,
        ):
            eoall_t = {}

            def combine(mc):
                eoall = eoall_t[mc]
                for sb in range(SB):
                    ps_o = psO.tile([P, MCH], F32, tag="pso", name=f"pso{mc}_{sb}")
                    nc.tensor.matmul(
                        ps_o[:],
                        lhsT=cwT_sb[:, sb, :],
                        rhs=eoall[:],
                        start=True, stop=True,
                    )
                    o_sb = outp.tile([P, MCH], F32, tag="osb", name=f"o{mc}_{sb}")
                    if sb % 8 < 5:
                        nc.vector.tensor_copy(o_sb[:], ps_o[:])
                    else:
                        nc.scalar.copy(o_sb[:], ps_o[:])
                    if sb % 4 == 1:
                        nc.scalar.dma_start(out[ts(sb, P), ts(mc, MCH)], o_sb[:])
                    else:
                        nc.sync.dma_start(out[ts(sb, P), ts(mc, MCH)], o_sb[:])

            FFN2_ENG = [0, 1, 2, 0] * 4

            def cast_mc(mc):
                wob_t = []
                for hq in range(16):
                    q, idx = divmod(hq, 4)
                    wt = wob_pool.tile([P, 4, MCH], BF16, tag="wob", name=f"wob{mc}_{hq}")
                    eng = (nc.vector.tensor_copy, nc.scalar.copy, nc.gpsimd.tensor_copy)[FFN2_ENG[hq]]
                    eng(wt[:], wo8_t[(mc, q)][:, ts(idx, 4), :])
                    wob_t.append(wt)
                return wob_t

            wob_all = {0: cast_mc(0)}
            for mc in range(MC):
                if mc + 1 < MC:
                    wob_all[mc + 1] = cast_mc(mc + 1)
                wob_t = wob_all[mc]
                ps_eo = psE.tile([E, MCH], F32, tag="pse", name=f"pseo{mc}")
                for ho in range(HB):
                    hq, j = divmod(ho, 4)
                    nc.tensor.matmul(
                        ps_eo[:], lhsT=ht_sb[:, ho, :], rhs=wob_t[hq][:, j, :],
                        start=(ho == 0), stop=(ho == HB - 1),
                    )
                eo_sb = sp3.tile([E, MCH], BF16, tag="eosb", name=f"eo{mc}")
                nc.vector.tensor_copy(eo_sb[:], ps_eo[:])
                nc.sync.dma_start(eo_in[mc][:], eo_sb[:])
                nc.gpsimd.collective_compute(
                    "AllToAll", ALU.bypass, replica_groups=rg,
                    ins=[eo_in[mc].opt()], outs=[eo_out[mc].opt()],
                )
                eoall = sp3.tile([E, MCH], BF16, tag="eoall", name=f"eoall{mc}")
                nc.sync.dma_start(eoall[:], eo_out[mc][:])
                eoall_t[mc] = eoall
                if mc > 0:
                    combine(mc - 1)
            combine(MC - 1)

    nc.finalize()
    return nc


def prepare_in_maps(x, Wg, bg, wi, wo):
    G, S, M = x.shape
    E, H, _ = wi.shape
    MO, HB = M // P, H // P
    E3 = ml_dtypes.float8_e3m4
    wg_arr = np.ascontiguousarray(
        np.asarray(Wg, dtype=np.float32).reshape(MO, P, E).transpose(1, 0, 2)
    )
    bg_arr = np.ascontiguousarray(np.asarray(bg, dtype=np.float32).reshape(1, E))
    in_maps = []
    for c in range(N_CORES):
        wiT = np.asarray(wi[c], dtype=np.float32).T * SCALE_WI      # [M, H]
        wi8_c = np.ascontiguousarray(
            wiT.reshape(MO, P, NSUP, HSUP).transpose(2, 1, 0, 3)
        ).astype(E3)                                                # [NSUP,P,MO,HSUP]
        woT = np.asarray(wo[c], dtype=np.float32).T * SCALE_WO      # [H, M]
        wo8_c = np.ascontiguousarray(
            woT.reshape(4, HB // 4, P, MC, MCH).transpose(3, 0, 2, 1, 4)
        ).astype(E3)                                                # [MC,4,P,HB/4,MCH]
        in_maps.append({
            "xg": np.ascontiguousarray(x[c], dtype=np.float32),
            "wg": wg_arr,
            "bgp": bg_arr,
            "wi8": wi8_c,
            "wo8": wo8_c,
        })
    return in_maps


def kernel(x, Wg, bg, wi, wo):
    global LAST_RESULT
    from concourse.bass_utils import run_bass_kernel_spmd

    x = np.asarray(x); Wg = np.asarray(Wg); bg = np.asarray(bg)
    wi = np.asarray(wi); wo = np.asarray(wo)
    nc = build_bass()
    in_maps = prepare_in_maps(x, Wg, bg, wi, wo)
    try:
        res = run_bass_kernel_spmd(
            nc, in_maps, core_ids=list(range(N_CORES)),
            trace=bool(int(os.environ.get("MOE_TRACE", "0"))),
        )
    except ModuleNotFoundError:
        # NTFF profiling hook unavailable in this environment — run untraced.
        os.environ["BASS_NEVER_TRACE"] = "1"
        res = run_bass_kernel_spmd(nc, in_maps, core_ids=list(range(N_CORES)))
    LAST_RESULT = res
    out = np.stack([r["out"] for r in res.results]).astype(np.float32)
    return out


# revision 27
# speedup vs baseline: 1.0046x; 1.0046x over previous
# MoE layer (top-2 routing, degenerate capacity C=2) on 8 TRN2 NeuronCores.
#
# Math (the reference collapses the capacity axis: both slots carry identical
# values, so the combine contributes a factor 2):
#   scores = softmax(x @ Wg + bg)                      [G,S,E]
#   top-2 per token -> dm (0/1 mask), cw = 2 * softmax(top2 probs) scattered
#   D[e,g,:]  = sum_s dm[g,s,e] * x[g,s,:]             (dispatch, per group)
#   h[e,g,:]  = silu(D[e,g,:] @ wi[e].T)
#   eo[e,g,:] = h[e,g,:] @ wo[e].T
#   out[g,s,:] = 2 * sum_e cw[g,s,e] * eo[e,g,:]
#
# Sharding: core c owns group g=c for gating/dispatch/combine and expert e=c
# for the FFN; two small AllToAlls (E==G==n_cores==8) move the [8,M]-sized
# dispatched rows / expert outputs between the two phases.
#
# Perf structure (cost-model driven):
#  - Gating runs in exact fp32 (PE transposes of x + fp32 score matmuls);
#    top-2 selection is bit-compatible with the fp32 reference path.
#  - FFN weights are stored in HBM as float8_e3m4 scaled by 64/128 (halves
#    the dominant DMA traffic); they are upcast fp8->bf16 on-chip by the
#    DVE/Act/Pool engines (cheap there), and all matmuls run in bf16.
#    The scales fold into the silu (scale=1/64) and the combine weights
#    (cw *= 2/8192), costing zero extra instructions.
#  - x is loaded once (fp32); the bf16 copy for the dispatch matmul is cast
#    on-chip instead of a second HBM read.
#  - Weight DMAs are 8 (wi) + 4 (wo) giant stripe transfers with contiguous
#    per-partition runs; interleaved with x tiles on the sync queue so the
#    DMA engines never idle.

import os
from contextlib import ExitStack

import numpy as np
import ml_dtypes

import concourse.bass as bass
from concourse import bacc
import concourse.mybir as mybir
import concourse.tile as tile
from concourse.bass import ts
from concourse.masks import make_identity

F32 = mybir.dt.float32
BF16 = mybir.dt.bfloat16
F8E3 = mybir.dt.float8e3
AF = mybir.ActivationFunctionType
ALU = mybir.AluOpType
AX = mybir.AxisListType

P = 128

G_FULL, S_FULL, M_FULL, H_FULL, E_FULL = 8, 2048, 2048, 8192, 8
N_CORES = 8

SCALE_WI = 64.0       # host multiplies wi by this before fp8 quantization
SCALE_WO = 128.0      # host multiplies wo by this before fp8 quantization
CW_SCALE = 2.0 / (SCALE_WI * SCALE_WO)   # capacity factor 2 / both scales

NSUP, HSUP = 16, 512  # wi stripes along H (one 512-wide h-chunk per stripe)
MC, MCH = 4, 512      # FFN2 / combine / output m-chunks

LAST_RESULT = None  # BassKernelResults of the most recent device run (for test.py)


def build_bass(S=S_FULL, M=M_FULL, H=H_FULL, E=E_FULL, n_cores=N_CORES):
    assert E == n_cores
    SB, MO, HB = S // P, M // P, H // P
    HC = HSUP // 512

    nc = bacc.Bacc(num_devices=n_cores)
    rg = [list(range(n_cores))]

    xg = nc.declare_dram_parameter("xg", [S, M], F32, False)
    wg = nc.declare_dram_parameter("wg", [P, MO, E], F32, False)
    bgp = nc.declare_dram_parameter("bgp", [1, E], F32, False)
    wi8 = nc.declare_dram_parameter("wi8", [NSUP, P, MO, HSUP], F8E3, False)
    wo8 = nc.declare_dram_parameter("wo8", [MC, 4, P, HB // 4, MCH], F8E3, False)
    out = nc.declare_dram_parameter("out", [S, M], F32, True)

    with tile.TileContext(nc) as tc, ExitStack() as stack:
        # ---------- persistent pools ----------
        const_pool = stack.enter_context(tc.tile_pool(name="const", bufs=1))
        ident_f = const_pool.tile([P, P], F32, name="ident_f")
        make_identity(nc, ident_f)
        ident_b = const_pool.tile([P, P], BF16, name="ident_b")
        nc.vector.tensor_copy(ident_b[:], ident_f[:])
        ones1 = const_pool.tile([1, P], F32, name="ones1")
        nc.vector.memset(ones1[:], 1.0)
        wg_sb = const_pool.tile([P, MO, E], F32, name="wg_sb")
        nc.scalar.dma_start(wg_sb[:], wg[:])
        bg_sb = const_pool.tile([1, E], F32, name="bg_sb")
        nc.scalar.dma_start(bg_sb[:], bgp[:])

        keep_pool = stack.enter_context(tc.tile_pool(name="keep", bufs=1))
        cwT_sb = keep_pool.tile([E, SB, P], BF16, name="cwT_sb")
        dt_sb = keep_pool.tile([P, MO, E], BF16, name="dt_sb")
        ht_sb = keep_pool.tile([P, HB, E], BF16, name="ht_sb")

        # weight streaming pools (created early; DMAs emitted at chosen points)
        wo8_pool = stack.enter_context(tc.tile_pool(name="wo8", bufs=12))
        wi8_ctx = tc.tile_pool(name="wi8", bufs=2)
        wi8_pool = wi8_ctx.__enter__()

        dram = stack.enter_context(tc.tile_pool(name="dram", bufs=1, space="DRAM"))
        d_in = [dram.tile([E, M], BF16, name=f"d_in{i}") for i in range(2)]
        d_out = [dram.tile([E, M], BF16, name=f"d_out{i}") for i in range(2)]
        eo_in = [dram.tile([E, MCH], BF16, name=f"eo_in{i}") for i in range(MC)]
        eo_out = [dram.tile([E, MCH], BF16, name=f"eo_out{i}") for i in range(MC)]

        wi8_t = {}

        def fetch_wi(hs):
            t = wi8_pool.tile([P, MO, HSUP], F8E3, tag="wi8", name=f"wi8_{hs}")
            nc.sync.dma_start(t[:], wi8[hs, :, :, :])
            wi8_t[hs] = t

        wo8_t = {}

        def fetch_wo(mc, q):
            t = wo8_pool.tile([P, HB // 4, MCH], F8E3, tag="wo8", name=f"wo8_{mc}_{q}")
            nc.sync.dma_start(t[:], wo8[mc, q, :, :, :])
            wo8_t[(mc, q)] = t

        # ---------- phase A: gating + dispatch (group-parallel) ----------
        with (
            tc.tile_pool(name="xa", bufs=3) as xa,
            tc.tile_pool(name="xt", bufs=2) as xt,
            tc.tile_pool(name="sp", bufs=2) as sp,
            tc.tile_pool(name="psA", bufs=2, space="PSUM") as psA,
            tc.tile_pool(name="psC", bufs=2, space="PSUM") as psC,
            tc.tile_pool(name="psD", bufs=1, space="PSUM") as psD,
        ):
            d_ps = psD.tile([E, M], F32, name="d_ps")
            # warmups: absorb const-pool ticks into PE's vector clock so the
            # hot-loop matmuls carry few semaphore waits each.
            ptd = psA.tile([P, 4 * P], F32, tag="pt", bufs=2, name="ptd")
            nc.tensor.transpose(ptd[:, 0:P], ident_f[:], ident_f[:])
            dmy0 = psA.tile([E, E], F32, tag="score", bufs=1, name="dmy0")
            nc.tensor.matmul(dmy0[:], lhsT=wg_sb[:, 0, :], rhs=wg_sb[:, 0, :], start=True, stop=True)
            dmyc = psC.tile([P, P], BF16, tag="pc", bufs=1, name="dmyc")
            nc.tensor.transpose(dmyc[:], ident_b[:], ident_b[:])

            for sb in range(SB):
                x_t = xa.tile([P, M], F32, tag="x", name=f"x{sb}")
                nc.sync.dma_start(x_t[:], xg[ts(sb, P), :])
                xT_t = xt.tile([P, M], F32, tag="xT", name=f"xT{sb}")
                for q in range(4):
                    pt = psA.tile([P, 4 * P], F32, tag="pt", bufs=2, name=f"pt{sb}_{q}")
                    for j in range(4):
                        mo = q * 4 + j
                        nc.tensor.transpose(pt[:, ts(j, P)], x_t[:, ts(mo, P)], ident_f[:])
                    if q % 2 == 0:
                        nc.vector.tensor_copy(xT_t[:, ts(q, 4 * P)], pt[:])
                    else:
                        nc.scalar.copy(xT_t[:, ts(q, 4 * P)], pt[:])
                # bf16 x for the dispatch matmul (cast on-chip; saves an HBM read)
                x_b = xa.tile([P, M], BF16, tag="xb", name=f"xb{sb}")
                if sb % 2 == 0:
                    nc.vector.tensor_copy(x_b[:], x_t[:])
                else:
                    nc.gpsimd.tensor_copy(x_b[:], x_t[:])

                score_ps = psA.tile([P, E], F32, tag="score", bufs=1, name=f"score{sb}")
                for mo in range(MO):
                    nc.tensor.matmul(
                        score_ps[:], lhsT=xT_t[:, ts(mo, P)], rhs=wg_sb[:, mo, :],
                        start=(mo == 0), stop=False,
                    )
                nc.tensor.matmul(
                    score_ps[:], lhsT=ones1[:], rhs=bg_sb[:], start=False, stop=True,
                )

                # top-2 gating in [tokens, E] layout (fp32 throughout)
                rm = sp.tile([P, 1], F32, tag="rm", name=f"rm{sb}")
                nc.vector.tensor_reduce(rm[:], score_ps[:], axis=AX.X, op=ALU.max, negate=True)
                probs = sp.tile([P, E], F32, tag="probs", name=f"probs{sb}")
                sume = sp.tile([P, 1], F32, tag="sume", name=f"sume{sb}")
                nc.scalar.activation(probs[:], score_ps[:], AF.Exp, bias=rm[:], accum_out=sume[:])
                rcp = sp.tile([P, 1], F32, tag="rcp", name=f"rcp{sb}")
                nc.vector.reciprocal(rcp[:], sume[:])
                pn = sp.tile([P, E], F32, tag="pn", name=f"pn{sb}")
                nc.vector.tensor_scalar_mul(pn[:], probs[:], rcp[:])
                p1 = sp.tile([P, 1], F32, tag="p1", name=f"p1{sb}")
                nc.vector.tensor_reduce(p1[:], pn[:], axis=AX.X, op=ALU.max)
                oh1 = sp.tile([P, E], F32, tag="oh1", name=f"oh1{sb}")
                nc.vector.tensor_scalar(oh1[:], pn[:], p1[:], None, op0=ALU.is_equal)
                pm = sp.tile([P, E], F32, tag="pm", name=f"pm{sb}")
                nc.gpsimd.tensor_tensor(pm[:], pn[:], oh1[:], ALU.subtract)
                p2 = sp.tile([P, 1], F32, tag="p2", name=f"p2{sb}")
                nc.vector.tensor_reduce(p2[:], pm[:], axis=AX.X, op=ALU.max)
                oh2 = sp.tile([P, E], F32, tag="oh2", name=f"oh2{sb}")
                nc.vector.tensor_scalar(oh2[:], pm[:], p2[:], None, op0=ALU.is_equal)
                e1 = sp.tile([P, 1], F32, tag="e1", name=f"e1{sb}")
                nc.scalar.activation(e1[:], p1[:], AF.Exp)
                e2 = sp.tile([P, 1], F32, tag="e2", name=f"e2{sb}")
                nc.scalar.activation(e2[:], p2[:], AF.Exp)
                s12 = sp.tile([P, 1], F32, tag="s12", name=f"s12{sb}")
                nc.vector.tensor_tensor(s12[:], e1[:], e2[:], ALU.add)
                r12 = sp.tile([P, 1], F32, tag="r12", name=f"r12{sb}")
                nc.vector.reciprocal(r12[:], s12[:])
                # CW_SCALE folds the capacity factor 2 and both weight scales
                w1 = sp.tile([P, 1], F32, tag="w1", name=f"w1{sb}")
                nc.vector.tensor_scalar(w1[:], e1[:], r12[:], CW_SCALE, op0=ALU.mult, op1=ALU.mult)
                w2 = sp.tile([P, 1], F32, tag="w2", name=f"w2{sb}")
                nc.vector.tensor_scalar(w2[:], e2[:], r12[:], CW_SCALE, op0=ALU.mult, op1=ALU.mult)
                cw_t = sp.tile([P, E], F32, tag="cw", name=f"cw{sb}")
                nc.vector.tensor_scalar_mul(cw_t[:], oh1[:], w1[:])
                t2 = sp.tile([P, E], F32, tag="t2", name=f"t2{sb}")
                nc.vector.tensor_scalar_mul(t2[:], oh2[:], w2[:])
                nc.vector.tensor_tensor(cw_t[:], cw_t[:], t2[:], ALU.add)
                # dispatch mask in bf16 (values 0/1 exact)
                dm_b = sp.tile([P, E], BF16, tag="dmb", name=f"dmb{sb}")
                nc.gpsimd.tensor_tensor(dm_b[:], oh1[:], oh2[:], ALU.add)

                # cw^T (bf16) into [E, S] layout for the combine matmul
                cw_b = sp.tile([P, E], BF16, tag="cwb", name=f"cwb{sb}")
                nc.vector.tensor_copy(cw_b[:], cw_t[:])
                pc = psC.tile([P, P], BF16, tag="pc", bufs=1, name=f"pc{sb}")
                nc.tensor.transpose(pc[:E, :], cw_b[:], ident_b[:])
                nc.vector.tensor_copy(cwT_sb[:, sb, :], pc[:E, :])

                # dispatch: D[e,m] += dm[s,e]^T @ x[s,m]  (bf16, accumulated over sb)
                for c in range(4):
                    nc.tensor.matmul(
                        d_ps[:, ts(c, MCH)],
                        lhsT=dm_b[:],
                        rhs=x_b[:, ts(c, MCH)],
                        start=(sb == 0), stop=(sb == SB - 1),
                    )

            # queue weight stripes behind the x tiles; mc0's wo quarters fill
            # the DMA gap while later wi stripes wait for their pool slots
            # (freed only as FFN1 casts consume them).
            for hs in range(4):
                fetch_wi(hs)
            for q in range(4):
                fetch_wo(0, q)

            # dispatch AllToAll: row e -> core e; receive [G, M] for my expert
            d_sb = sp.tile([E, M], BF16, tag="dsb", bufs=1, name="d_sb")
            nc.vector.tensor_copy(d_sb[:, 0:M // 2], d_ps[:, 0:M // 2])
            nc.scalar.copy(d_sb[:, M // 2:M], d_ps[:, M // 2:M])
            nc.gpsimd.dma_start(d_in[0][:], d_sb[:])
            nc.gpsimd.collective_compute(
                "AllToAll", ALU.bypass, replica_groups=rg,
                ins=[d_in[0].opt()], outs=[d_out[0].opt()],
            )
            for hs in range(4, NSUP):
                fetch_wi(hs)
            de_b = sp.tile([E, M], BF16, tag="deb", bufs=1, name="de_b")
            nc.scalar.dma_start(de_b[:], d_out[0][:])
            for mo in range(MO):
                pd = psC.tile([P, E], BF16, tag="pc", bufs=1, name=f"pd{mo}")
                nc.tensor.transpose(pd[:], de_b[:, ts(mo, P)], ident_b[:E, :E])
                if mo % 2 == 0:
                    nc.vector.tensor_copy(dt_sb[:, mo, :], pd[:])
                else:
                    nc.scalar.copy(dt_sb[:, mo, :], pd[:])

        for mc in range(MC):
            for q in range(4):
                if (mc, q) not in wo8_t:
                    fetch_wo(mc, q)

        # ---------- phase B1: expert FFN1 + silu (expert-parallel) ----------
        cast_rr = 0

        def cast(dst, src):
            nonlocal cast_rr
            eng = (nc.vector.tensor_copy, nc.scalar.copy, nc.gpsimd.tensor_copy)[cast_rr % 3]
            cast_rr += 1
            eng(dst, src)

        with (
            tc.tile_pool(name="wib", bufs=4) as wib_pool,
            tc.tile_pool(name="sp2", bufs=2) as sp2,
            tc.tile_pool(name="psB", bufs=4, space="PSUM") as psB,
            tc.tile_pool(name="psH", bufs=2, space="PSUM") as psH,
        ):
            dmy1 = psB.tile([E, E], F32, tag="psh", name="dmy1")
            nc.tensor.matmul(dmy1[:], lhsT=dt_sb[:, MO - 1, :], rhs=dt_sb[:, MO - 1, :], start=True, stop=True)
            FFN1_ENG = [0, 1, 2, 0, 2, 1, 0, 2, 0, 1, 2, 0, 1, 2, 0, 0]
            for hs in range(NSUP):
                wib_t = wib_pool.tile([P, MO, HSUP], BF16, tag="wib", name=f"wib{hs}")
                for mo in range(MO):
                    eng = (nc.vector.tensor_copy, nc.scalar.copy, nc.gpsimd.tensor_copy)[FFN1_ENG[mo]]
                    eng(wib_t[:, mo, :], wi8_t[hs][:, mo, :])
                hc = hs
                ps_h = psB.tile([E, MCH], F32, tag="psh", name=f"psh{hc}")
                for mo in range(MO):
                    nc.tensor.matmul(
                        ps_h[:], lhsT=dt_sb[:, mo, :],
                        rhs=wib_t[:, mo, :],
                        start=(mo == 0), stop=(mo == MO - 1),
                    )
                hf = sp2.tile([E, MCH], F32, tag="hf", name=f"hf{hc}")
                nc.vector.tensor_copy(hf[:], ps_h[:])
                sg = sp2.tile([E, MCH], F32, tag="sg", name=f"sg{hc}")
                nc.scalar.activation(sg[:], hf[:], AF.Sigmoid, scale=1.0 / SCALE_WI)
                h_sb = sp2.tile([E, MCH], BF16, tag="hsb", name=f"h{hc}")
                nc.vector.tensor_tensor(h_sb[:], hf[:], sg[:], ALU.mult)
                pht = psH.tile([P, 4, E], BF16, tag="pht", name=f"pht{hc}")
                for j in range(4):
                    nc.tensor.transpose(pht[:, j, :], h_sb[:, ts(j, P)], ident_b[:E, :E])
                if hc % 2 == 0:
                    nc.vector.tensor_copy(ht_sb[:, 4 * hc:4 * (hc + 1), :], pht[:])
                else:
                    nc.scalar.copy(ht_sb[:, 4 * hc:4 * (hc + 1), :], pht[:])

        wi8_ctx.__exit__(None, None, None)

        # ---------- phase B2: FFN2 + AllToAll + combine, pipelined per m-chunk ----------
        with (
            tc.tile_pool(name="wob", bufs=20) as wob_pool,
            tc.tile_pool(name="sp3", bufs=3) as sp3,
            tc.tile_pool(name="outp", bufs=6) as outp,
            tc.tile_pool(name="psE", bufs=2, space="PSUM") as psE,
            tc.tile_pool(name="psO", bufs=4, space="PSUM") as psO,
        ):
            eoall_t = {}

            def combine(mc):
                eoall = eoall_t[mc]
                for sb in range(SB):
                    ps_o = psO.tile([P, MCH], F32, tag="pso", name=f"pso{mc}_{sb}")
                    nc.tensor.matmul(
                        ps_o[:],
                        lhsT=cwT_sb[:, sb, :],
                        rhs=eoall[:],
                        start=True, stop=True,
                    )
                    o_sb = outp.tile([P, MCH], F32, tag="osb", name=f"o{mc}_{sb}")
                    if sb % 8 < 5:
                        nc.vector.tensor_copy(o_sb[:], ps_o[:])
                    else:
                        nc.scalar.copy(o_sb[:], ps_o[:])
                    if sb % 4 == 1:
                        nc.scalar.dma_start(out[ts(sb, P), ts(mc, MCH)], o_sb[:])
                    else:
                        nc.sync.dma_start(out[ts(sb, P), ts(mc, MCH)], o_sb[:])

            FFN2_ENG = [0, 1, 2, 0, 2, 1, 0, 2, 0, 1, 2, 0, 1, 2, 0, 0]

            def cast_mc(mc):
                wob_t = []
                for hq in range(16):
                    q, idx = divmod(hq, 4)
                    wt = wob_pool.tile([P, 4, MCH], BF16, tag="wob", name=f"wob{mc}_{hq}")
                    eng = (nc.vector.tensor_copy, nc.scalar.copy, nc.gpsimd.tensor_copy)[FFN2_ENG[hq]]
                    eng(wt[:], wo8_t[(mc, q)][:, ts(idx, 4), :])
                    wob_t.append(wt)
                return wob_t

            wob_all = {0: cast_mc(0)}
            for mc in range(MC):
                if mc + 1 < MC:
                    wob_all[mc + 1] = cast_mc(mc + 1)
                wob_t = wob_all[mc]
                ps_eo = psE.tile([E, MCH], F32, tag="pse", name=f"pseo{mc}")
                for ho in range(HB):
                    hq, j = divmod(ho, 4)
                    nc.tensor.matmul(
                        ps_eo[:], lhsT=ht_sb[:, ho, :], rhs=wob_t[hq][:, j, :],
                        start=(ho == 0), stop=(ho == HB - 1),
                    )
                eo_sb = sp3.tile([E, MCH], BF16, tag="eosb", name=f"eo{mc}")
                nc.vector.tensor_copy(eo_sb[:], ps_eo[:])
                nc.gpsimd.dma_start(eo_in[mc][:], eo_sb[:])
                nc.gpsimd.collective_compute(
                    "AllToAll", ALU.bypass, replica_groups=rg,
                    ins=[eo_in[mc].opt()], outs=[eo_out[mc].opt()],
                )
                eoall = sp3.tile([E, MCH], BF16, tag="eoall", name=f"eoall{mc}")
                nc.gpsimd.dma_start(eoall[:], eo_out[mc][:])
                eoall_t[mc] = eoall
                if mc > 0:
                    combine(mc - 1)
            combine(MC - 1)

    nc.finalize()
    return nc


def prepare_in_maps(x, Wg, bg, wi, wo):
    G, S, M = x.shape
    E, H, _ = wi.shape
    MO, HB = M // P, H // P
    E3 = ml_dtypes.float8_e3m4
    wg_arr = np.ascontiguousarray(
        np.asarray(Wg, dtype=np.float32).reshape(MO, P, E).transpose(1, 0, 2)
    )
    bg_arr = np.ascontiguousarray(np.asarray(bg, dtype=np.float32).reshape(1, E))
    in_maps = []
    for c in range(N_CORES):
        wiT = np.asarray(wi[c], dtype=np.float32).T * SCALE_WI      # [M, H]
        wi8_c = np.ascontiguousarray(
            wiT.reshape(MO, P, NSUP, HSUP).transpose(2, 1, 0, 3)
        ).astype(E3)                                                # [NSUP,P,MO,HSUP]
        woT = np.asarray(wo[c], dtype=np.float32).T * SCALE_WO      # [H, M]
        wo8_c = np.ascontiguousarray(
            woT.reshape(4, HB // 4, P, MC, MCH).transpose(3, 0, 2, 1, 4)
        ).astype(E3)                                                # [MC,4,P,HB/4,MCH]
        in_maps.append({
            "xg": np.ascontiguousarray(x[c], dtype=np.float32),
            "wg": wg_arr,
            "bgp": bg_arr,
            "wi8": wi8_c,
            "wo8": wo8_c,
        })
    return in_maps


def kernel(x, Wg, bg, wi, wo):
    global LAST_RESULT
    from concourse.bass_utils import run_bass_kernel_spmd

    x = np.asarray(x); Wg = np.asarray(Wg); bg = np.asarray(bg)
    wi = np.asarray(wi); wo = np.asarray(wo)
    nc = build_bass()
    in_maps = prepare_in_maps(x, Wg, bg, wi, wo)
    try:
        res = run_bass_kernel_spmd(
            nc, in_maps, core_ids=list(range(N_CORES)),
            trace=bool(int(os.environ.get("MOE_TRACE", "0"))),
        )
    except ModuleNotFoundError:
        # NTFF profiling hook unavailable in this environment — run untraced.
        os.environ["BASS_NEVER_TRACE"] = "1"
        res = run_bass_kernel_spmd(nc, in_maps, core_ids=list(range(N_CORES)))
    LAST_RESULT = res
    out = np.stack([r["out"] for r in res.results]).astype(np.float32)
    return out
